# revision 8
# baseline (speedup 1.0000x reference)
"""Trainium2 Bass kernel for nn_Attention_3719441678662.

ViT-style attention block (B=16, N=577 tokens, C=768, H=12 heads, D=64)
with a CLS-row reweighting before softmax and a second output carrying the
pre-softmax CLS->patch scores.

Distribution: data-parallel over batch. 8 NeuronCores x 2 batches each;
weights replicated. Each core computes its two batches fully on-chip.

Per-core dataflow (all matmul operands bf16, fp32 PSUM accumulation):
  x --cast dma--> DRAM bf16 --xbar-transpose dma--> xT [C, N] sbuf
  qkT[f, n] = w_qkv[:, f].T @ xT          (f in q|k halves, 12 x 128)
  v[n, f]   = xT[:, ntile].T @ w_qkv_v    (token-major, + ones column)
  scoresT[m, n] = kT.T @ qT               (per head, K=64 row-pair packed)
  patch/reweight on scoresT[:, 0] column; exp via ScalarE (scale=1/8)
  outT[d, n] = v_aug.T @ expT             (row 64 = softmax denominator)
  out = (outAT.T @ w_proj + b) * recip(denom)  per token partition
"""

import sys

sys.path.insert(0, "/opt/trn_rl_repo")

import numpy as np

B, N, C = 16, 577, 768
H, D = 12, 64
SCALE = D ** -0.5
ALPHA = 0.1
NCORES = 8
BPC = B // NCORES  # batches per core

KT = C // 128  # 6 k-tiles over the C contraction
NPAD = 640  # token dim padded for the xbar dma-transpose (mult of 128)
# token tiles (also used as score m-tiles): 4 x 128 + 65
TOK_TILES = [(i * 128, min(128, N - i * 128)) for i in range((N + 127) // 128)]
# free-dim chunks of the token axis for matmul N<=512 (one PSUM bank fp32)
NCHUNKS = [(0, 512), (512, N - 512)]
# free-dim chunks of a 768-wide output
CCHUNKS = [(0, 512), (512, 256)]


def _build():
    import concourse.bass as bass
    import concourse.tile as tile
    from concourse import bacc, mybir
    from concourse.masks import make_identity

    f32 = mybir.dt.float32
    bf16 = mybir.dt.bfloat16

    nc = bacc.Bacc(None, target_bir_lowering=False, debug=False)

    x_d = nc.dram_tensor("x", [BPC, N, C], f32, kind="ExternalInput")
    aw_d = nc.dram_tensor("attn_weight", [BPC, N - 1], f32, kind="ExternalInput")
    wqkv_d = nc.dram_tensor("w_qkv", [C, 3 * C], f32, kind="ExternalInput")
    bqkv_d = nc.dram_tensor("b_qkv", [3 * C], f32, kind="ExternalInput")
    wproj_d = nc.dram_tensor("w_proj", [C, C], f32, kind="ExternalInput")
    bproj_d = nc.dram_tensor("b_proj", [C], f32, kind="ExternalInput")
    out_d = nc.dram_tensor("out", [BPC, N, C], f32, kind="ExternalOutput")
    patch_d = nc.dram_tensor("patch_attn", [BPC, H, N - 1], f32, kind="ExternalOutput")

    from contextlib import ExitStack

    with tile.TileContext(nc) as tc, ExitStack() as ctx:
        singles = ctx.enter_context(tc.tile_pool(name="singles", bufs=1))
        dram = ctx.enter_context(tc.tile_pool(name="dram", bufs=1, space="DRAM"))
        xT_p = ctx.enter_context(tc.tile_pool(name="xT", bufs=2 * KT))
        qkT_p = ctx.enter_context(tc.tile_pool(name="qkT", bufs=2 * 12))
        vaug_p = ctx.enter_context(tc.tile_pool(name="vaug", bufs=2 * len(TOK_TILES)))
        expT_p = ctx.enter_context(tc.tile_pool(name="expT", bufs=2))
        outAT_p = ctx.enter_context(tc.tile_pool(name="outAT", bufs=2 * KT))
        outsb_p = ctx.enter_context(tc.tile_pool(name="outsb", bufs=3))
        small_p = ctx.enter_context(tc.tile_pool(name="small", bufs=4))
        row_p = ctx.enter_context(tc.tile_pool(name="rows", bufs=4))
        bc_p = ctx.enter_context(tc.tile_pool(name="bc", bufs=4))
        mm_p = ctx.enter_context(tc.tile_pool(name="mm", bufs=4, space="PSUM"))
        pair_p = ctx.enter_context(tc.tile_pool(name="pair", bufs=1, space="PSUM"))

        # ---- constants / weights staging ----
        wqkv_sb = singles.tile([128, KT, 3 * C], bf16)
        for k in range(KT):
            nc.gpsimd.dma_start(
                out=wqkv_sb[:, k, :], in_=wqkv_d[k * 128 : (k + 1) * 128, :]
            )
        wproj_sb = singles.tile([128, KT, C], bf16)
        for k in range(KT):
            nc.gpsimd.dma_start(
                out=wproj_sb[:, k, :], in_=wproj_d[k * 128 : (k + 1) * 128, :]
            )
        # b_qkv for the q|k halves as per-partition columns [128, 12]
        bqk_cols = singles.tile([128, 12], f32)
        nc.sync.dma_start(
            out=bqk_cols, in_=bqkv_d[0 : 12 * 128].rearrange("(t p) -> p t", p=128)
        )
        bv_row = singles.tile([1, C], bf16)  # v-part bias as a row
        nc.gpsimd.dma_start(out=bv_row, in_=bqkv_d[2 * C : 3 * C].rearrange("(a c) -> a c", a=1))
        bproj_row = singles.tile([1, C], bf16)
        nc.gpsimd.dma_start(out=bproj_row, in_=bproj_d.rearrange("(a c) -> a c", a=1))
        ones_bf = singles.tile([1, 128], bf16)
        nc.vector.memset(ones_bf, 1.0)
        ident = singles.tile([128, 128], f32)
        make_identity(nc, ident)
        zrow = singles.tile([NPAD - N, C], bf16)
        nc.vector.memset(zrow, 0.0)

        x_bf = dram.tile([BPC, NPAD, C], bf16)

        for b in range(BPC):
            # ---- stage x: cast to bf16 in DRAM, pad, dma-transpose to xT ----
            nc.gpsimd.dma_start(out=x_bf[b, 0:N, :], in_=x_d[b])
            nc.sync.dma_start(out=x_bf[b, N:NPAD, :], in_=zrow)
            xT = []
            for k in range(KT):
                t = xT_p.tile([128, NPAD], bf16)
                nc.sync.dma_start(
                    out=t, in_=x_bf[b, :, k * 128 : (k + 1) * 128], transpose=True
                )
                xT.append(t)

            # ---- attn_weight -> per-partition reweight factors [128, 5] ----
            wcol = small_p.tile([128, len(TOK_TILES)], f32, tag="wcol")
            nc.vector.memset(wcol, 0.0)
            nc.gpsimd.dma_start(out=wcol[1:128, 0:1], in_=aw_d[b, 0:127])
            nc.gpsimd.dma_start(
                out=wcol[:, 1:4],
                in_=aw_d[b, 127:511].rearrange("(t p) -> p t", p=128),
            )
            nc.gpsimd.dma_start(out=wcol[0:65, 4:5], in_=aw_d[b, 511:576])
            factor = small_p.tile([128, len(TOK_TILES)], f32, tag="factor")
            nc.vector.tensor_scalar(
                out=factor,
                in0=wcol,
                scalar1=ALPHA,
                scalar2=1.0 - ALPHA,
                op0=mybir.AluOpType.mult,
                op1=mybir.AluOpType.add,
            )
            nc.vector.memset(factor[0:1, 0:1], 1.0)  # CLS->CLS is not reweighted

            # ---- qkT: [feat 128-tile, token] for q (tiles 0-5) and k (6-11) ----
            qkT = []
            for mt in range(12):
                qk_t = qkT_p.tile([128, N], bf16)
                for c0, clen in NCHUNKS:
                    ps = mm_p.tile([128, clen], f32, tag="mm")
                    for k in range(KT):
                        nc.tensor.matmul(
                            ps,
                            wqkv_sb[:, k, mt * 128 : (mt + 1) * 128],
                            xT[k][:, c0 : c0 + clen],
                            start=(k == 0),
                            stop=(k == KT - 1),
                        )
                    nc.any.tensor_scalar_add(
                        out=qk_t[:, c0 : c0 + clen],
                        in0=ps,
                        scalar1=bqk_cols[:, mt : mt + 1],
                    )
                qkT.append(qk_t)

            # ---- v token-major with ones column: v_aug [tok, 12*(64+1)] ----
            vaug = []
            for t0, tlen in TOK_TILES:
                va = vaug_p.tile([128, H, D + 1], bf16)
                nc.vector.memset(va[0:tlen, :, D : D + 1], 1.0)
                for c0, clen in CCHUNKS:
                    ps = mm_p.tile([128, clen], f32, tag="mm")
                    for k in range(KT):
                        nc.tensor.matmul(
                            ps[0:tlen],
                            xT[k][:, t0 : t0 + tlen],
                            wqkv_sb[:, k, 2 * C + c0 : 2 * C + c0 + clen],
                            start=(k == 0),
                            stop=False,
                        )
                    nc.tensor.matmul(
                        ps[0:tlen],
                        ones_bf[:, 0:tlen],
                        bv_row[:, c0 : c0 + clen],
                        start=False,
                        stop=True,
                    )
                    h0, h1 = c0 // D, (c0 + clen) // D
                    nc.any.tensor_copy(
                        out=va[0:tlen, h0:h1, 0:D],
                        in_=ps[0:tlen].rearrange("p (h d) -> p h d", d=D),
                    )
                vaug.append(va)

            # ---- attention, head pairs ----
            patchbuf = small_p.tile([128, H, len(TOK_TILES)], f32, tag="patch")
            outAT = []
            for j in range(KT):
                expT = expT_p.tile([128, len(TOK_TILES), 2, N], bf16)
                for it, (t0, tlen) in enumerate(TOK_TILES):
                    pp = pair_p.tile([128, 2, 1024], f32)
                    for hh in range(2):
                        for c0, clen in NCHUNKS:
                            nc.tensor.matmul(
                                pp[0:tlen, hh, c0 : c0 + clen],
                                qkT[6 + j][hh * 64 : hh * 64 + 64, t0 : t0 + tlen],
                                qkT[j][hh * 64 : hh * 64 + 64, c0 : c0 + clen],
                                start=True,
                                stop=True,
                            )
                    # pre-softmax CLS->patch scores (column n=0), then reweight
                    nc.vector.tensor_scalar_mul(
                        out=patchbuf[0:tlen, 2 * j : 2 * j + 2, it : it + 1],
                        in0=pp[0:tlen, :, 0:1],
                        scalar1=SCALE,
                    )
                    nc.vector.tensor_scalar_mul(
                        out=pp[0:tlen, :, 0:1],
                        in0=pp[0:tlen, :, 0:1],
                        scalar1=factor[0:tlen, it : it + 1],
                    )
                    nc.scalar.activation(
                        out=expT[0:tlen, it, :, :],
                        in_=pp[0:tlen, :, 0:N],
                        func=mybir.ActivationFunctionType.Exp,
                        scale=SCALE,
                    )

                oa = outAT_p.tile([128, N], bf16)
                for hh in range(2):
                    h = 2 * j + hh
                    avs = []
                    rr = row_p.tile([1, N], f32, tag="rr")
                    for c0, clen in NCHUNKS:
                        av = mm_p.tile([65, clen], f32, tag="mm")
                        for it, (t0, tlen) in enumerate(TOK_TILES):
                            nc.tensor.matmul(
                                av,
                                vaug[it][0:tlen, h, :],
                                expT[0:tlen, it, hh, c0 : c0 + clen],
                                start=(it == 0),
                                stop=(it == len(TOK_TILES) - 1),
                            )
                        nc.vector.reciprocal(
                            out=rr[:, c0 : c0 + clen], in_=av[D : D + 1, :]
                        )
                        avs.append(av)
                    rr_dram = dram.tile([N], f32, tag="rrd", bufs=4)
                    nc.sync.dma_start(out=rr_dram, in_=rr)
                    bc = bc_p.tile([64, N], f32)
                    rr_b = bass.AP(
                        tensor=rr_dram.tensor,
                        offset=rr_dram.offset,
                        ap=[[0, 64]] + list(rr_dram.ap),
                    )
                    nc.sync.dma_start(out=bc, in_=rr_b)
                    for (c0, clen), av in zip(NCHUNKS, avs):
                        nc.vector.tensor_mul(
                            out=oa[hh * 64 : hh * 64 + 64, c0 : c0 + clen],
                            in0=av[0:D, :],
                            in1=bc[:, c0 : c0 + clen],
                        )
                outAT.append(oa)

            # ---- patch_attn: transpose [tok, H] tiles -> [H, tok], dma out ----
            patchT = small_p.tile([12, NPAD], f32, tag="patchT")
            for it, (t0, tlen) in enumerate(TOK_TILES):
                pt = mm_p.tile([12, 128], f32, tag="mm")
                nc.tensor.transpose(
                    pt[:, 0:tlen], patchbuf[0:tlen, :, it], ident[0:tlen, 0:tlen]
                )
                nc.any.tensor_copy(out=patchT[:, t0 : t0 + tlen], in_=pt[:, 0:tlen])
            nc.sync.dma_start(out=patch_d[b], in_=patchT[:, 1:N])

            # ---- proj + bias, per token tile ----
            for t0, tlen in TOK_TILES:
                osb = outsb_p.tile([128, C], f32)
                for c0, clen in CCHUNKS:
                    ps = mm_p.tile([128, clen], f32, tag="mm")
                    for k in range(KT):
                        nc.tensor.matmul(
                            ps[0:tlen],
                            outAT[k][:, t0 : t0 + tlen],
                            wproj_sb[:, k, c0 : c0 + clen],
                            start=(k == 0),
                            stop=False,
                        )
                    nc.tensor.matmul(
                        ps[0:tlen],
                        ones_bf[:, 0:tlen],
                        bproj_row[:, c0 : c0 + clen],
                        start=False,
                        stop=True,
                    )
                    nc.any.tensor_copy(out=osb[0:tlen, c0 : c0 + clen], in_=ps[0:tlen])
                nc.sync.dma_start(out=out_d[b, t0 : t0 + tlen, :], in_=osb[0:tlen])

    return nc


_STATE = {}


def _get_nc():
    if "nc" not in _STATE:
        nc = _build()
        if not nc.is_finalized():
            nc.finalize()
        _STATE["nc"] = nc
    return _STATE["nc"]


def _get_exec():
    """Build (once) a cached jitted shard_map executable over the 8 cores."""
    if "exec" in _STATE:
        return _STATE["exec"]

    import jax
    from jax.experimental.shard_map import shard_map
    from jax.sharding import Mesh, NamedSharding, PartitionSpec

    from concourse import bass2jax, mybir

    bass2jax.install_neuronx_cc_hook()
    nc = _get_nc()

    in_names, out_names, out_avals = [], [], []
    part_name = nc.partition_id_tensor.name if nc.partition_id_tensor else None
    for alloc in nc.m.functions[0].allocations:
        if not isinstance(alloc, mybir.MemoryLocationSet):
            continue
        name = alloc.memorylocations[0].name
        if alloc.kind == "ExternalInput":
            if name != part_name:
                in_names.append(name)
        elif alloc.kind == "ExternalOutput":
            out_names.append(name)
            out_avals.append(
                jax.core.ShapedArray(
                    tuple(alloc.tensor_shape), mybir.dt.np(alloc.dtype)
                )
            )
    n_params = len(in_names)
    all_in_names = list(in_names) + list(out_names)
    if part_name is not None:
        all_in_names.append(part_name)

    def _body(*args):
        operands = list(args)
        if part_name is not None:
            operands.append(bass2jax.partition_id_tensor())
        outs = bass2jax._bass_exec_p.bind(
            *operands,
            out_avals=tuple(out_avals),
            in_names=tuple(all_in_names),
            out_names=tuple(out_names),
            lowering_input_output_aliases=(),
            sim_require_finite=True,
            sim_require_nnan=True,
            nc=nc,
        )
        return tuple(outs)

    devices = jax.devices()[:NCORES]
    mesh = Mesh(np.asarray(devices), ("core",))
    n_outs = len(out_names)
    donate = tuple(range(n_params, n_params + n_outs))
    sharded = jax.jit(
        shard_map(
            _body,
            mesh=mesh,
            in_specs=(PartitionSpec("core"),) * (n_params + n_outs),
            out_specs=(PartitionSpec("core"),) * n_outs,
            check_rep=False,
        ),
        donate_argnums=donate,
        keep_unused=True,
    )
    _STATE["exec"] = {
        "fn": sharded,
        "in_names": in_names,
        "out_names": out_names,
        "out_avals": out_avals,
        "sharding": NamedSharding(mesh, PartitionSpec("core")),
    }
    return _STATE["exec"]


def _concat_inputs(x, attn_weight, w_qkv, b_qkv, w_proj, b_proj):
    """Global (8*shard) arrays in executable input order."""
    f = lambda a: np.ascontiguousarray(np.asarray(a, dtype=np.float32))
    per_name = {
        "x": f(x),
        "attn_weight": f(attn_weight),
        "w_qkv": np.concatenate([f(w_qkv)] * NCORES, axis=0),
        "b_qkv": np.concatenate([f(b_qkv)] * NCORES, axis=0),
        "w_proj": np.concatenate([f(w_proj)] * NCORES, axis=0),
        "b_proj": np.concatenate([f(b_proj)] * NCORES, axis=0),
    }
    ex = _get_exec()
    return [per_name[name] for name in ex["in_names"]]


def _zero_outs():
    ex = _get_exec()
    return [
        np.zeros((NCORES * a.shape[0], *a.shape[1:]), a.dtype) for a in ex["out_avals"]
    ]


def kernel(x, attn_weight, w_qkv, b_qkv, w_proj, b_proj):
    ex = _get_exec()
    ins = _concat_inputs(x, attn_weight, w_qkv, b_qkv, w_proj, b_proj)
    outs = ex["fn"](*ins, *_zero_outs())
    res = {name: np.asarray(o) for name, o in zip(ex["out_names"], outs)}
    return res["out"], res["patch_attn"]
